# revision 36
# baseline (speedup 1.0000x reference)
"""AFNB (asymmetric fusion non-local block) Trainium2 kernel, 8 NeuronCores.

Sharding: core c handles batch b = c//2, spatial row-half s = c%2 (36 of 72
rows). No collectives: each core computes the pooled key (which needs the full
spatial extent of its batch element) by running the low-feats conv over all 72
rows; queries / fused output only over its own 36-row half.

Host algebra (exact, fp64): BatchNorm folded everywhere; fusion conv split
wf @ [ctx; high] = Wcc@attn + wbh@high with Wcc = (wf_c*sf)@ww; value path
pre-pooled on host (pooling commutes with the 1x1 conv).

Precision: the fused conv's high part runs KF8 of 16 contraction k-tiles in
fp8-e4m3 DoubleRow (2 k-tiles per matmul instruction -> half the instructions)
and the rest in bf16. All products in one PSUM group at scale 4096 (fp8: acts
x8, weights x512; bf16: weights x4096) so fp8 operands sit mid-range (no
subnormals); the output op applies a uniform 1/4096.

The fp8 rounding uses host-side GPTQ-style sequential compensation over the
full 2048-row contraction: fp8 rows are rounded first and their quantization
error is propagated into the remaining rows, which ship in bf16 and so absorb
it near-exactly. Both the weights (vs the core's actual activations) and the
activations (vs the quantized weights) are compensated per core. Measured
rel err ~1.75e-2 at KF8=10 (limit 2e-2).

Schedule: warmup matmuls keep the PE P-state up while inputs stream (its
result store is enqueued after all input DMAs so the Sync engine never blocks
input transfers behind it); chunk-0 query conv + k-outer fused high-parts of
4 output tiles hide the key-conv/pooling/softmax serial chain; steady loop
runs one chunk of fused output while the next chunk's attention chain and the
chunk-after-next's DMAs proceed.
"""

import numpy as np
import ml_dtypes

B, CL, CH, CK, CV, CO = 4, 1024, 2048, 256, 256, 2048
H = W = 72
N = H * W          # 5184
HL = 36            # rows per core
NL = HL * W        # 2592
PSP = (1, 3, 6, 8)
M = 110            # pooled locations
MP = 112           # padded fp8 stride
EPS = 1e-5
NCORES = 8
KL, KH = CL // 128, CH // 128   # 8, 16 contraction tiles
KF8 = 10                         # fused-conv k-tiles in fp8 (even)
KBH = KH - KF8                   # bf16 fused-conv k-tiles
NF8 = KF8 * 128
NT = 432                         # moving-dim tile (6 rows of 72)
NTILES_A = N // NT               # 12
NCH_B = NL // NT                 # 6

SX = 8.0          # fp8 activation scale
SWL = 128.0       # fp8 low-conv weight scale
SWQ = 256.0       # fp8 query-conv weight scale
SWH = 512.0       # fp8 fused weight scale (products at SX*SWH = 4096)
SPS = SX * SWH    # fused PSUM scale
SK8 = 16.0        # fp8 pooled-key scale
SQ8 = 16.0        # fp8 query scale
SCC = 16.0        # fp8 ctx scale (wcc at SPS/SCC = 256)

F8 = ml_dtypes.float8_e4m3
BF = ml_dtypes.bfloat16

_CACHE = {}
LAST_RESULTS = None


def _build_graph():
    import concourse.bacc as bacc
    import concourse.mybir as mybir
    from concourse import tile, masks

    F32 = mybir.dt.float32
    BF16 = mybir.dt.bfloat16
    FP8 = mybir.dt.float8e4
    AF = mybir.ActivationFunctionType
    AX = mybir.AxisListType
    ALU = mybir.AluOpType
    DR = mybir.MatmulPerfMode.DoubleRow

    nc = bacc.Bacc("TRN2", target_bir_lowering=False, debug=False, num_devices=NCORES)

    low_d = nc.dram_tensor("low", [128, NTILES_A, KL, NT], FP8,
                           kind="ExternalInput").ap()
    h8r_d = nc.dram_tensor("h8r", [128, NCH_B, KBH, NT], FP8,
                           kind="ExternalInput").ap()
    hq8_d = nc.dram_tensor("hq8", [128, NCH_B, KF8, NT], FP8,
                           kind="ExternalInput").ap()
    hb_d = nc.dram_tensor("hb", [128, NCH_B, KBH, NT], BF16,
                          kind="ExternalInput").ap()
    wlow_d = nc.dram_tensor("wlow", [128, 2, KL, 128], FP8, kind="ExternalInput").ap()
    wv_d = nc.dram_tensor("wv", [128, KL, 256], BF16, kind="ExternalInput").ap()
    psp_d = nc.dram_tensor("psp", [128, KL, MP], BF16, kind="ExternalInput").ap()
    wq_d = nc.dram_tensor("wq", [128, KH, 256], FP8, kind="ExternalInput").ap()
    wbhb_d = nc.dram_tensor("wbhb", [128, KBH, 2048], BF16, kind="ExternalInput").ap()
    wbh8_d = nc.dram_tensor("wbh8", [128, KF8, 2048], FP8, kind="ExternalInput").ap()
    wcc_d = nc.dram_tensor("wcc", [128, 2, CO], FP8, kind="ExternalInput").ap()
    blow_d = nc.dram_tensor("blow", [128, 2], F32, kind="ExternalInput").ap()
    bq_d = nc.dram_tensor("bq", [128, 2], F32, kind="ExternalInput").ap()
    bout_d = nc.dram_tensor("bout", [128, 16], F32, kind="ExternalInput").ap()
    binv_d = nc.dram_tensor("binv", [128, 2 * M], F32, kind="ExternalInput").ap()
    out_d = nc.dram_tensor("out", [128, NCH_B, 16, NT], BF16,
                           kind="ExternalOutput").ap()
    warm_d = nc.dram_tensor("warm", [1, 4], F32, kind="ExternalOutput").ap()

    with tile.TileContext(nc) as tc:
        with (
            tc.tile_pool(name="wp", bufs=1) as wp,
            tc.tile_pool(name="cp", bufs=1) as cp,
            tc.tile_pool(name="lp", bufs=8) as lp,
            tc.tile_pool(name="cvp", bufs=4) as cvp,
            tc.tile_pool(name="gp", bufs=1) as gp,
            tc.tile_pool(name="pp", bufs=1) as pp,
            tc.tile_pool(name="hp", bufs=3) as hp,
            tc.tile_pool(name="h8p", bufs=3) as h8p,
            tc.tile_pool(name="hq8p", bufs=3) as hq8p,
            tc.tile_pool(name="ab", bufs=2) as ab,
            tc.tile_pool(name="op", bufs=3) as op_,
            tc.tile_pool(name="ps", bufs=3, space="PSUM") as ps,
            tc.tile_pool(name="psO", bufs=5, space="PSUM") as psO,
        ):
            # ---- PE warmup: keeps the PE P-state up during input DMA ----
            warm_in = cp.tile([128, NT], BF16, name="warm_in")
            nc.vector.memset(warm_in, 0.0)
            warm_ps = ps.tile([128, NT], F32, name="warm_ps", tag="ps")
            for _ in range(46):
                nc.tensor.matmul(warm_ps, warm_in[:, :128], warm_in,
                                 start=True, stop=True)
            warm_sb = cp.tile([1, 4], F32, name="warm_sb")
            nc.scalar.copy(warm_sb, warm_ps[:1, :4])

            # ---- phase-A weights: ct=0 slice first ----
            wlow_sb = wp.tile([128, 2 * KL * 128], FP8, name="wlow_sb").rearrange(
                "p (c k m) -> p c k m", c=2, k=KL)
            nc.sync.dma_start(out=wlow_sb[:, 0], in_=wlow_d[:, 0])
            blow_sb = cp.tile([128, 2], F32, name="blow_sb")
            nc.sync.dma_start(out=blow_sb, in_=blow_d)
            wv_sb = wp.tile([128, KL * 256], BF16, name="wv_sb").rearrange(
                "p (k m) -> p k m", k=KL)
            psp_sb = wp.tile([128, KL * MP], BF16, name="psp_sb").rearrange(
                "p (k m) -> p k m", k=KL)

            wq_sb = wp.tile([128, KH * 256], FP8, name="wq_sb").rearrange(
                "p (k m) -> p k m", k=KH)
            wbhb_sb = wp.tile([128, KBH * 2048], BF16, name="wbhb_sb").rearrange(
                "p (k m) -> p k m", k=KBH)
            wbh8_sb = wp.tile([128, KF8 * 2048], FP8, name="wbh8_sb").rearrange(
                "p (k m) -> p k m", k=KF8)
            wcc_sb = wp.tile([128, 2 * CO], FP8, name="wcc_sb").rearrange(
                "p (k m) -> p k m", k=2)
            st = {}  # per-chunk live tiles

            def load_high(c):
                hc = hp.tile([128, KBH * NT], BF16, name="high_c",
                             tag="high_c").rearrange("p (k n) -> p k n", k=KBH)
                nc.sync.dma_start(out=hc, in_=hb_d[:, c])
                hcq = hq8p.tile([128, KF8 * NT], FP8, name="hq8_c",
                                tag="hq8_c").rearrange("p (k n) -> p k n", k=KF8)
                nc.sync.dma_start(out=hcq, in_=hq8_d[:, c])
                st.setdefault(c, {}).update(high=hc, hq8=hcq)

            def load_h8r(c):
                hc8 = h8p.tile([128, KBH * NT], FP8, name="h8r_c",
                               tag="h8r_c").rearrange("p (k n) -> p k n", k=KBH)
                nc.sync.dma_start(out=hc8, in_=h8r_d[:, c])
                st.setdefault(c, {})['h8r'] = hc8

            # ---- phase A: key conv (256 ch, relu) over full spatial ----
            # grid columns: ct-major [ct(2) | gi(24) | gj(24)]
            gridall = gp.tile([128, 2 * 576], F32, name="gridall")

            def conv_ct(nt, ct, low_c):
                cps = ps.tile([128, NT], F32, name="cv_ps", tag="ps")
                for t in range(KL // 2):
                    nc.tensor.matmul(cps, wlow_sb[:, ct, 2 * t:2 * t + 2, :],
                                     low_c[:, 2 * t:2 * t + 2, :],
                                     start=(t == 0), stop=(t == KL // 2 - 1),
                                     perf_mode=DR)
                cv = cvp.tile([128, NT], F32, name="cv", tag="cv")
                nc.scalar.activation(cv, cps, AF.Relu,
                                     bias=blow_sb[:, ct:ct + 1],
                                     scale=1.0 / (SX * SWL))
                # 6 rows x 72 cols -> 2 grid-rows x 24 grid-cols (3x3 cells)
                cvv = cv.rearrange("p (gi ri gj cj) -> p gi gj ri cj",
                                   gi=2, ri=3, gj=24, cj=3)
                gsl = gridall[:, ct * 576 + nt * 48: ct * 576 + (nt + 1) * 48
                              ].rearrange("p (gi gj) -> p gi gj", gi=2)
                for gi in range(2):
                    nc.vector.reduce_sum(gsl[:, gi], cvv[:, gi], axis=AX.XY)

            def extra_dma(nt):
                if nt == 0:
                    nc.sync.dma_start(out=wlow_sb[:, 1], in_=wlow_d[:, 1])
                    nc.sync.dma_start(out=wv_sb, in_=wv_d)
                    nc.sync.dma_start(out=psp_sb, in_=psp_d)
                    bq_sb = cp.tile([128, 2], F32, name="bq_sb")
                    nc.sync.dma_start(out=bq_sb, in_=bq_d)
                    bout_sb = cp.tile([128, 16], F32, name="bout_sb")
                    nc.sync.dma_start(out=bout_sb, in_=bout_d)
                    binv_sb = cp.tile([128, 2 * M], F32, name="binv_sb")
                    nc.sync.dma_start(out=binv_sb, in_=binv_d)
                    cdict['bq'], cdict['bout'], cdict['binv'] = bq_sb, bout_sb, binv_sb
                elif nt == 1:
                    load_high(0)
                    load_h8r(0)
                elif nt == 2:
                    nc.sync.dma_start(out=wq_sb, in_=wq_d)
                    nc.sync.dma_start(out=wbh8_sb[:, 0:2], in_=wbh8_d[:, 0:2])
                elif nt == 3:
                    nc.sync.dma_start(out=wbh8_sb[:, 2:4], in_=wbh8_d[:, 2:4])
                    nc.sync.dma_start(out=wbhb_sb[:, 0], in_=wbhb_d[:, 0])
                elif nt == 4:
                    nc.sync.dma_start(out=wcc_sb, in_=wcc_d)
                    # warmup result store: enqueued only now, after all
                    # latency-critical input DMAs, so Sync never stalls on it
                    nc.sync.dma_start(out=warm_d, in_=warm_sb)
                    nc.sync.dma_start(out=wbh8_sb[:, 4:6], in_=wbh8_d[:, 4:6])
                    nc.sync.dma_start(out=wbhb_sb[:, 1], in_=wbhb_d[:, 1])
                elif nt == 5:
                    nc.sync.dma_start(out=wbh8_sb[:, 6:8], in_=wbh8_d[:, 6:8])
                    nc.sync.dma_start(out=wbhb_sb[:, 2], in_=wbhb_d[:, 2])
                elif nt == 6:
                    nc.sync.dma_start(out=wbh8_sb[:, 8:10], in_=wbh8_d[:, 8:10])
                    nc.sync.dma_start(out=wbhb_sb[:, 3], in_=wbhb_d[:, 3])
                elif nt == 7:
                    nc.sync.dma_start(out=wbhb_sb[:, 4], in_=wbhb_d[:, 4])
                elif nt == 8:
                    nc.sync.dma_start(out=wbhb_sb[:, 5], in_=wbhb_d[:, 5])

            cdict = {}
            low_tiles = {}

            def load_low(nt):
                low_c = lp.tile([128, KL * NT], FP8, name="low_c",
                                tag="low_c").rearrange("p (k n) -> p k n", k=KL)
                nc.sync.dma_start(out=low_c, in_=low_d[:, nt])
                low_tiles[nt] = low_c

            for nt in range(7):
                load_low(nt)

            # ---- value path: small bf16 matmuls on the host-pooled input ----
            v_sc = [pp.tile([128, M], F32, name=f"vsc{t}") for t in range(2)]

            def emit_value():
                for ct in range(2):
                    vps = ps.tile([128, MP], F32, name="v_ps", tag="ps")
                    for t in range(KL):
                        nc.tensor.matmul(vps,
                                         wv_sb[:, t, ct * 128:(ct + 1) * 128],
                                         psp_sb[:, t, :],
                                         start=(t == 0), stop=(t == KL - 1))
                    nc.vector.tensor_mul(v_sc[ct], vps[:, 0:110],
                                         cdict['binv'][:, M:2 * M])

            def emit_q(c):
                # query conv reads the GPTQ fp8 tiles (0..KF8) plus an RTN
                # fp8 copy of the bf16 tiles -- no separate full-16 tensor
                if 'q' in st[c]:
                    return
                hcq, hr = st[c]['hq8'], st[c]['h8r']
                q_sb = ab.tile([128, 2 * NT], FP8, name="q_sb",
                               tag="q_sb").rearrange("p (t n) -> p t n", t=2)
                for qt in range(2):
                    qps = ps.tile([128, NT], F32, name="q_ps", tag="ps")
                    for t in range(KF8 // 2):
                        nc.tensor.matmul(qps,
                                         wq_sb[:, 2 * t:2 * t + 2,
                                               qt * 128:(qt + 1) * 128],
                                         hcq[:, 2 * t:2 * t + 2, :],
                                         start=(t == 0), stop=False,
                                         perf_mode=DR)
                    for t in range(KBH // 2):
                        nc.tensor.matmul(qps,
                                         wq_sb[:, KF8 + 2 * t:KF8 + 2 * t + 2,
                                               qt * 128:(qt + 1) * 128],
                                         hr[:, 2 * t:2 * t + 2, :],
                                         start=False, stop=(t == KBH // 2 - 1),
                                         perf_mode=DR)
                    nc.scalar.activation(q_sb[:, qt], qps, AF.Relu,
                                         bias=cdict['bq'][:, qt:qt + 1],
                                         scale=0.0625 * SQ8 / (SX * SWQ))
                st[c]['q'] = q_sb

            # ---- chunk-0 fused high-parts run inside phase A; each finished
            # PSUM accumulation is spilled to SBUF bf16 so the banks rotate.
            # After the attention chain, a cheap bf16-identity matmul reloads
            # each spill and the wcc ctx pair finishes the group. ----
            acc0 = wp.tile([128, 16 * NT], BF16, name="acc0").rearrange(
                "p (ct n) -> p ct n", ct=16)

            def c0_high(cts):
                hc0, hcq0 = st[0]['high'], st[0]['hq8']
                tiles = {}
                for ct in cts:
                    ops = psO.tile([128, NT], F32, name="o_ps", tag="psO")
                    for t in range(KF8 // 2):
                        nc.tensor.matmul(ops,
                                         wbh8_sb[:, 2 * t:2 * t + 2,
                                                 ct * 128:(ct + 1) * 128],
                                         hcq0[:, 2 * t:2 * t + 2, :],
                                         start=(t == 0), stop=False,
                                         perf_mode=DR)
                    tiles[ct] = ops
                for ct in cts:
                    ops = tiles[ct]
                    for k in range(KBH):
                        nc.tensor.matmul(ops,
                                         wbhb_sb[:, k, ct * 128:(ct + 1) * 128],
                                         hc0[:, k, :], start=False,
                                         stop=(k == KBH - 1))
                    nc.scalar.copy(acc0[:, ct], ops)

            def c0_fin(cts):
                tiles = {}
                for ct in cts:
                    ops = psO.tile([128, NT], F32, name="o_ps", tag="psO")
                    nc.tensor.matmul(ops, ident_bf, acc0[:, ct],
                                     start=True, stop=False)
                    tiles[ct] = ops
                for ct in cts:
                    ops = tiles[ct]
                    nc.tensor.matmul(ops, wcc_sb[:, :, ct * 128:(ct + 1) * 128],
                                     st[0]['ctx'], start=False, stop=True,
                                     perf_mode=DR)
                    emit_out(0, ct, ops)

            for nt in range(NTILES_A):
                if nt + 7 < NTILES_A:
                    load_low(nt + 7)
                low_c = low_tiles.pop(nt)
                if nt == 0:
                    conv_ct(0, 0, low_c)
                    extra_dma(0)
                    ident = cp.tile([128, 128], F32, name="ident")
                    masks.make_identity(nc, ident)
                    ident_bf = cp.tile([128, 128], BF16, name="ident_bf")
                    masks.make_identity(nc, ident_bf)
                    # 1/16 here folds the SCC=16 ctx scale into the softmax
                    # normalizer: r = 16/sum(E)
                    ones110 = cp.tile([110, 1], BF16, name="ones110")
                    nc.vector.memset(ones110, 1.0 / SCC)
                    ones1 = cp.tile([1, 128], BF16, name="ones1")
                    nc.vector.memset(ones1, 1.0)
                    conv_ct(nt, 1, low_c)
                    continue
                extra_dma(nt)
                conv_ct(nt, 0, low_c)
                conv_ct(nt, 1, low_c)
                if nt == 3:
                    emit_value()
                elif nt == 4:
                    emit_q(0)
                elif nt == 8:
                    c0_high(range(0, 2))
                elif nt == 9:
                    c0_high(range(2, 4))
                    load_h8r(1)
                elif nt == 10:
                    c0_high(range(4, 6))
                    load_high(1)
                elif nt == 11:
                    c0_high(range(6, 8))
                    load_h8r(2)
            c0_high(range(8, 12))
            load_high(2)
            c0_high(range(12, 16))
            bq_sb, bout_sb, binv_sb = cdict['bq'], cdict['bout'], cdict['binv']

            # ---- stage-2 pooling: grid -> 110 pooled key sums per tile ----
            pooled = [pp.tile([128, M], F32, name=f"pooled{t}") for t in range(2)]

            def stage2(t):
                g = gridall[:, t * 576:(t + 1) * 576]
                nc.vector.reduce_sum(pooled[t][:, 0:1], g, axis=AX.X)
                for s, off in ((3, 1), (6, 10), (8, 46)):
                    cs = 24 // s
                    gv = g.rearrange("p (bi ci bj cj) -> p bi bj ci cj",
                                     bi=s, ci=cs, bj=s, cj=cs)
                    for bi in range(s):
                        nc.vector.reduce_sum(
                            pooled[t][:, off + bi * s: off + (bi + 1) * s],
                            gv[:, bi], axis=AX.XY)

            k8 = pp.tile([128, 2 * MP], FP8, name="k8").rearrange(
                "p (t m) -> p t m", t=2)
            for t in range(2):
                stage2(t)
                nc.vector.tensor_mul(k8[:, t, 0:110], pooled[t], binv_sb[:, 0:M])
            VT = [pp.tile([110, 128], BF16, name=f"VT{t}") for t in range(2)]

            # ---- attention stages ----
            def emit_sim(c):
                if 'E' in st[c]:
                    return
                sim_ps = ps.tile([110, NT], F32, name="sim_ps", tag="ps")
                nc.tensor.matmul(sim_ps, k8[:, :, 0:110], st[c]['q'],
                                 start=True, stop=True, perf_mode=DR)
                E = ab.tile([110, NT], BF16, name="E", tag="E")
                nc.scalar.activation(E, sim_ps, AF.Exp, scale=1.0 / (SK8 * SQ8))
                st[c]['E'] = E

            def emit_s(c):
                if 'r' in st[c]:
                    return
                s_ps = ps.tile([1, NT], F32, name="s_ps", tag="ps")
                nc.tensor.matmul(s_ps, ones110, st[c]['E'], start=True, stop=True)
                s_sb = ab.tile([1, NT], F32, name="s_sb", tag="s_sb")
                nc.vector.tensor_copy(s_sb, s_ps)
                r_sb = ab.tile([1, NT], F32, name="r_sb", tag="r_sb")
                nc.vector.reciprocal_approx_fast(r_sb, s_sb)
                r_bf = ab.tile([1, NT], BF16, name="r_bf", tag="r_bf")
                nc.vector.tensor_copy(r_bf, r_sb)
                st[c]['r'] = r_bf

            def emit_rbc(c):
                rbc_ps = ps.tile([128, NT], F32, name="rbc_ps", tag="ps")
                nc.tensor.matmul(rbc_ps, ones1, st[c]['r'], start=True, stop=True)
                rbc_sb = ab.tile([128, NT], F32, name="rbc_sb", tag="rbc_sb")
                nc.vector.tensor_copy(rbc_sb, rbc_ps)
                st[c]['rbc'] = rbc_sb

            def emit_ctx(c):
                ctx8 = ab.tile([128, 2 * NT], FP8, name="ctx8",
                               tag="ctx8").rearrange("p (t n) -> p t n", t=2)
                for t in range(2):
                    cps = ps.tile([128, NT], F32, name="ctx_ps", tag="ps")
                    nc.tensor.matmul(cps, VT[t], st[c]['E'], start=True, stop=True)
                    nc.vector.tensor_mul(ctx8[:, t], cps, st[c]['rbc'])
                st[c]['ctx'] = ctx8

            def emit_out(c, ct, ops):
                j = ct % 4
                if j == 0:
                    st[c]['o4'] = op_.tile([128, 4 * NT], BF16, name="o4",
                                           tag="o4").rearrange(
                        "p (j n) -> p j n", j=4)
                o4 = st[c]['o4']
                nc.vector.tensor_scalar(o4[:, j], ops, 1.0 / SPS,
                                        bout_sb[:, ct:ct + 1],
                                        ALU.mult, ALU.add)
                if j == 3:
                    nc.sync.dma_start(
                        out=out_d[:, c, ct - 3:ct + 1, :], in_=o4)

            def emit_fused(c, cts):
                # grouped: all fp8-DR parts (incl. the wcc ctx pair, which is
                # ready in steady state) back-to-back, then all bf16 parts --
                # one bf16<->fp8 PE mode switch per group instead of per ct
                hc, hcq, ctx = st[c]['high'], st[c]['hq8'], st[c]['ctx']
                for ct in cts:
                    ops = psO.tile([128, NT], F32, name="o_ps", tag="psO")
                    nc.tensor.matmul(ops, wcc_sb[:, :, ct * 128:(ct + 1) * 128],
                                     ctx, start=True, stop=False, perf_mode=DR)
                    for t in range(KF8 // 2):
                        nc.tensor.matmul(ops,
                                         wbh8_sb[:, 2 * t:2 * t + 2,
                                                 ct * 128:(ct + 1) * 128],
                                         hcq[:, 2 * t:2 * t + 2, :],
                                         start=False, stop=False, perf_mode=DR)
                    st[c][f'ops{ct}'] = ops
                for ct in cts:
                    ops = st[c].pop(f'ops{ct}')
                    for k in range(KBH):
                        nc.tensor.matmul(ops,
                                         wbhb_sb[:, k, ct * 128:(ct + 1) * 128],
                                         hc[:, k, :], start=False,
                                         stop=(k == KBH - 1))
                    emit_out(c, ct, ops)

            # ---- chunk-0 attention chain; chunk-1 stages and the chunk-0
            # fin groups fill its serial latencies ----
            emit_sim(0)
            for t in range(2):
                tp = ps.tile([110, 128], F32, name="vt_ps", tag="ps")
                nc.tensor.transpose(tp, v_sc[t], ident)
                nc.scalar.copy(VT[t], tp)
            emit_s(0)
            emit_q(1)            # hides the reciprocal chain of chunk 0
            emit_rbc(0)
            emit_sim(1)
            emit_ctx(0)
            c0_fin(range(0, 4))
            emit_s(1)
            c0_fin(range(4, 8))
            emit_rbc(1)
            emit_q(2)
            c0_fin(range(8, 12))
            emit_ctx(1)
            c0_fin(range(12, 16))
            del st[0]

            # ---- steady pipeline (q runs one chunk ahead of the chain) ----
            for c in range(1, NCH_B):
                n = c + 1
                if c + 2 < NCH_B:
                    load_h8r(c + 2)
                    load_high(c + 2)
                if n < NCH_B:
                    if n + 1 < NCH_B:
                        emit_q(n + 1)
                    emit_fused(c, range(0, 4))
                    emit_sim(n)
                    emit_fused(c, range(4, 8))
                    emit_s(n)
                    emit_fused(c, range(8, 12))
                    emit_rbc(n)
                    emit_ctx(n)
                    emit_fused(c, range(12, 16))
                else:
                    for g in range(0, 16, 4):
                        emit_fused(c, range(g, g + 4))
                del st[c]

    nc.compile()
    return nc


def _pack_tiles(a2d, ktiles):
    """(ktiles*128, F) -> [128, ktiles, F] contiguous."""
    k128, F = a2d.shape
    assert k128 == ktiles * 128
    return np.ascontiguousarray(a2d.reshape(ktiles, 128, F).transpose(1, 0, 2))


def _chunk_major(a2d, ktiles, dtype):
    """(ktiles*128, NL) -> [128, NCH_B, ktiles, NT] contiguous."""
    k128, nl = a2d.shape
    a = a2d.reshape(ktiles, 128, NCH_B, NT)
    return np.ascontiguousarray(a.transpose(1, 2, 0, 3)).astype(dtype)


def _bf16(a):
    return np.ascontiguousarray(a).astype(BF)


def _fp8(a, scale):
    x = np.asarray(a, np.float32) * scale
    return np.clip(x, -224.0, 224.0).astype(F8)


def _q_fp8(v, scale, clip=224.0):
    return (np.clip(v * scale, -clip, clip).astype(F8).astype(np.float32)
            / np.float32(scale))


def _q_bf16(v):
    return v.astype(BF).astype(np.float32)


def _gptq_mixed(Wf, Hd, nf8, scale8, blocksize=128, damp=0.01):
    """Round Wf (K, O) rows [0,nf8) to the fp8 grid (at scale8) and the rest
    to bf16, sequentially compensating each row's rounding error via the
    remaining rows (GPTQ). Hd: damped XX^T (K, K) f64. Returns f32 raw-domain
    quantized values."""
    K, O = Wf.shape
    W = np.array(Wf, np.float32)
    L = np.linalg.cholesky(np.linalg.inv(Hd))
    U = np.ascontiguousarray(L.T, np.float32)
    Q = np.empty_like(W)
    for i0 in range(0, nf8, blocksize):
        i1 = min(i0 + blocksize, nf8)
        Wb = W[i0:i1].copy()
        Eb = np.empty_like(Wb)
        for j in range(i1 - i0):
            k = i0 + j
            w = Wb[j]
            qv = _q_fp8(w, scale8)
            Q[k] = qv
            e = (w - qv) / U[k, k]
            Eb[j] = e
            if j + 1 < i1 - i0:
                Wb[j + 1:] -= np.outer(U[k, i0 + j + 1:i1], e)
        if i1 < K:
            W[i1:] -= U[i0:i1, i1:].T @ Eb
    Q[nf8:] = _q_bf16(W[nf8:])
    return Q


def _damp(Hf32):
    Hd = Hf32.astype(np.float64)
    Hd[np.diag_indices(Hd.shape[0])] += np.mean(np.diag(Hd)) * 0.01
    return Hd


def _prep_consts(inputs):
    f64 = {k: np.asarray(v, np.float64) for k, v in inputs.items()}
    sk = f64['gk'] / np.sqrt(f64['vk'] + EPS)
    sq = f64['gq'] / np.sqrt(f64['vq'] + EPS)
    sf = f64['gf'] / np.sqrt(f64['vf'] + EPS)

    wk_f = f64['wk'] * sk[:, None]
    bk_f = (f64['bk'] - f64['mk']) * sk + f64['bek']
    wq_f = f64['wq'] * sq[:, None]
    bq_f = ((f64['bq'] - f64['mq']) * sq + f64['beq']) / 16.0 * SQ8

    wf_s = f64['wf'] * sf[:, None]
    A = wf_s[:, :CO]          # context part (2048, 2048)
    Bh = wf_s[:, CO:]         # high part (2048, 2048)
    Wcc = A @ f64['ww']       # (2048, 256)
    # bv passes through the softmax-weighted value average unchanged
    # (weights sum to 1), so it folds into the output bias via Wcc.
    bout = (A @ f64['bw'] + (f64['bf'] - f64['mf']) * sf + f64['bef']
            + Wcc @ f64['bv'])

    binsize = np.concatenate(
        [np.full(s * s, float((H // s) * (W // s))) for s in PSP])  # (110,)
    s_m = 8.0 / np.sqrt(binsize)          # per-column scale of psp sums
    binv2 = np.concatenate([SK8 / binsize, 1.0 / ((s_m / 8.0) * binsize)])

    return {
        'wlow': _fp8(np.ascontiguousarray(
            wk_f.T.reshape(KL, 128, 2, 128).transpose(1, 2, 0, 3)), SWL),
        'wv': _bf16(_pack_tiles(np.ascontiguousarray(f64['wv'].T), KL)),
        'wq': _fp8(_pack_tiles(np.ascontiguousarray(wq_f.T), KH), SWQ),
        'wcc': _fp8(_pack_tiles(np.ascontiguousarray(Wcc.T), 2), SPS / SCC),
        'blow': np.ascontiguousarray(bk_f.reshape(2, 128).T, dtype=np.float32),
        'bq': np.ascontiguousarray(bq_f.reshape(2, 128).T, dtype=np.float32),
        'bout': np.ascontiguousarray(bout.reshape(16, 128).T, dtype=np.float32),
        'binv': np.ascontiguousarray(
            np.broadcast_to(binv2, (128, 2 * M)), dtype=np.float32),
        '_s_m': s_m,
        '_Wfull': np.ascontiguousarray(Bh.T, np.float32),   # (CH, CO)
    }


def _make_in_maps(inputs):
    consts = _prep_consts(inputs)
    s_m = consts.pop('_s_m')
    Wfull = consts.pop('_Wfull')
    low_feats = np.asarray(inputs['low_feats'], np.float32)
    high_feats = np.asarray(inputs['high_feats'], np.float32)

    # host-side spatial-pyramid sums of the low features (the value path's
    # pooling commutes with its 1x1 conv), bf16 with per-column scales
    lowr = low_feats.reshape(B, CL, H, W).astype(np.float64)
    psums = np.concatenate(
        [lowr.reshape(B, CL, s, H // s, s, W // s).sum(axis=(3, 5))
         .reshape(B, CL, s * s) for s in PSP], axis=2)       # (B, CL, 110)
    pspq = np.zeros((B, CL, MP), np.float64)
    pspq[:, :, :M] = psums * (s_m / 8.0)

    in_maps = []
    for c in range(NCORES):
        b, s = c // 2, c % 2
        low2d = low_feats[b].reshape(CL, N)
        X = np.ascontiguousarray(
            high_feats[b, :, s * HL:(s + 1) * HL, :].reshape(CH, NL))

        # GPTQ-mixed rounding: W vs this core's RTN-quantized X, then X vs
        # the quantized W; bf16 rows absorb the fp8 rows' rounding error.
        Xq_rtn = np.concatenate([_q_fp8(X[:NF8], SX), _q_bf16(X[NF8:])])
        Wg = _gptq_mixed(Wfull, _damp(Xq_rtn @ Xq_rtn.T), NF8, SWH)
        Xg = _gptq_mixed(X, _damp(Wg @ Wg.T), NF8, SX)

        m = dict(consts)
        m['low'] = np.ascontiguousarray(
            _fp8(_pack_tiles(low2d, KL), SX)          # [128, KL, N]
            .reshape(128, KL, NTILES_A, NT).transpose(0, 2, 1, 3))
        m['h8r'] = _chunk_major(_fp8(X[NF8:], SX), KBH, F8)
        m['hq8'] = _chunk_major(_fp8(Xg[:NF8], SX), KF8, F8)
        m['hb'] = _chunk_major(Xg[NF8:], KBH, BF)
        m['wbh8'] = _fp8(_pack_tiles(Wg[:NF8], KF8), SWH)
        m['wbhb'] = _bf16(_pack_tiles(Wg[NF8:] * SPS, KBH))
        m['psp'] = _bf16(_pack_tiles(pspq[b], KL))
        in_maps.append(m)
    return in_maps


def kernel(**inputs):
    global LAST_RESULTS
    from concourse.bass_utils import run_bass_kernel_spmd

    if 'nc' not in _CACHE:
        _CACHE['nc'] = _build_graph()
    nc = _CACHE['nc']

    res = run_bass_kernel_spmd(nc, _make_in_maps(inputs), list(range(NCORES)))
    LAST_RESULTS = res

    out = np.empty((B, CO, H, W), np.float32)
    for c in range(NCORES):
        b, s = c // 2, c % 2
        o = np.asarray(res.results[c]['out'], np.float32)  # [128, 6, 16, NT]
        o = o.transpose(2, 0, 1, 3).reshape(CO, HL, W)
        out[b, :, s * HL:(s + 1) * HL, :] = o
    return out


# revision 39
# speedup vs baseline: 1.0017x; 1.0017x over previous
"""AFNB (asymmetric fusion non-local block) Trainium2 kernel, 8 NeuronCores.

Sharding: core c handles batch b = c//2, spatial row-half s = c%2 (36 of 72
rows). No collectives: each core computes the pooled key (which needs the full
spatial extent of its batch element) by running the low-feats conv over all 72
rows; queries / fused output only over its own 36-row half.

Host algebra (exact, fp64): BatchNorm folded everywhere; fusion conv split
wf @ [ctx; high] = Wcc@attn + wbh@high with Wcc = (wf_c*sf)@ww; value path
pre-pooled on host (pooling commutes with the 1x1 conv).

Precision: the fused conv's high part runs KF8 of 16 contraction k-tiles in
fp8-e4m3 DoubleRow (2 k-tiles per matmul instruction -> half the instructions)
and the rest in bf16. All products in one PSUM group at scale 4096 (fp8: acts
x8, weights x512; bf16: weights x4096) so fp8 operands sit mid-range (no
subnormals); the output op applies a uniform 1/4096.

The fp8 rounding uses host-side GPTQ-style sequential compensation over the
full 2048-row contraction: fp8 rows are rounded first and their quantization
error is propagated into the remaining rows, which ship in bf16 and so absorb
it near-exactly. Both the weights (vs the core's actual activations) and the
activations (vs the quantized weights) are compensated per core. Measured
rel err ~1.75e-2 at KF8=10 (limit 2e-2).

Schedule: warmup matmuls keep the PE P-state up while inputs stream (its
result store is enqueued after all input DMAs so the Sync engine never blocks
input transfers behind it); chunk-0 query conv + k-outer fused high-parts of
4 output tiles hide the key-conv/pooling/softmax serial chain; steady loop
runs one chunk of fused output while the next chunk's attention chain and the
chunk-after-next's DMAs proceed.
"""

import numpy as np
import ml_dtypes

B, CL, CH, CK, CV, CO = 4, 1024, 2048, 256, 256, 2048
H = W = 72
N = H * W          # 5184
HL = 36            # rows per core
NL = HL * W        # 2592
PSP = (1, 3, 6, 8)
M = 110            # pooled locations
MP = 112           # padded fp8 stride
EPS = 1e-5
NCORES = 8
KL, KH = CL // 128, CH // 128   # 8, 16 contraction tiles
KF8 = 10                         # fused-conv k-tiles in fp8 (even)
KBH = KH - KF8                   # bf16 fused-conv k-tiles
NF8 = KF8 * 128
NT = 432                         # moving-dim tile (6 rows of 72)
NTILES_A = N // NT               # 12
NCH_B = NL // NT                 # 6

SX = 8.0          # fp8 activation scale
SWL = 128.0       # fp8 low-conv weight scale
SWQ = 256.0       # fp8 query-conv weight scale
SWH = 512.0       # fp8 fused weight scale (products at SX*SWH = 4096)
SPS = SX * SWH    # fused PSUM scale
SK8 = 16.0        # fp8 pooled-key scale
SQ8 = 16.0        # fp8 query scale
SCC = 16.0        # fp8 ctx scale (wcc at SPS/SCC = 256)

F8 = ml_dtypes.float8_e4m3
BF = ml_dtypes.bfloat16

_CACHE = {}
LAST_RESULTS = None


def _build_graph():
    import concourse.bacc as bacc
    import concourse.mybir as mybir
    from concourse import tile, masks

    F32 = mybir.dt.float32
    BF16 = mybir.dt.bfloat16
    FP8 = mybir.dt.float8e4
    AF = mybir.ActivationFunctionType
    AX = mybir.AxisListType
    ALU = mybir.AluOpType
    DR = mybir.MatmulPerfMode.DoubleRow

    nc = bacc.Bacc("TRN2", target_bir_lowering=False, debug=False, num_devices=NCORES)

    low_d = nc.dram_tensor("low", [128, NTILES_A, KL, NT], FP8,
                           kind="ExternalInput").ap()
    h8r_d = nc.dram_tensor("h8r", [128, NCH_B, KBH, NT], FP8,
                           kind="ExternalInput").ap()
    hq8_d = nc.dram_tensor("hq8", [128, NCH_B, KF8, NT], FP8,
                           kind="ExternalInput").ap()
    hb_d = nc.dram_tensor("hb", [128, NCH_B, KBH, NT], BF16,
                          kind="ExternalInput").ap()
    wlow_d = nc.dram_tensor("wlow", [128, 2, KL, 128], FP8, kind="ExternalInput").ap()
    wv_d = nc.dram_tensor("wv", [128, KL, 256], BF16, kind="ExternalInput").ap()
    psp_d = nc.dram_tensor("psp", [128, KL, MP], BF16, kind="ExternalInput").ap()
    wq_d = nc.dram_tensor("wq", [128, KH, 256], FP8, kind="ExternalInput").ap()
    wbhb_d = nc.dram_tensor("wbhb", [128, KBH, 2048], BF16, kind="ExternalInput").ap()
    wbh8_d = nc.dram_tensor("wbh8", [128, KF8, 2048], FP8, kind="ExternalInput").ap()
    wcc_d = nc.dram_tensor("wcc", [128, 2, CO], FP8, kind="ExternalInput").ap()
    blow_d = nc.dram_tensor("blow", [128, 2], F32, kind="ExternalInput").ap()
    bq_d = nc.dram_tensor("bq", [128, 2], F32, kind="ExternalInput").ap()
    bout_d = nc.dram_tensor("bout", [128, 16], F32, kind="ExternalInput").ap()
    binv_d = nc.dram_tensor("binv", [128, 2 * M], F32, kind="ExternalInput").ap()
    out_d = nc.dram_tensor("out", [128, NCH_B, 16, NT], BF16,
                           kind="ExternalOutput").ap()
    warm_d = nc.dram_tensor("warm", [1, 4], F32, kind="ExternalOutput").ap()

    with tile.TileContext(nc) as tc:
        with (
            tc.tile_pool(name="wp", bufs=1) as wp,
            tc.tile_pool(name="cp", bufs=1) as cp,
            tc.tile_pool(name="lp", bufs=6) as lp,
            tc.tile_pool(name="cvp", bufs=4) as cvp,
            tc.tile_pool(name="gp", bufs=1) as gp,
            tc.tile_pool(name="pp", bufs=1) as pp,
            tc.tile_pool(name="hp", bufs=3) as hp,
            tc.tile_pool(name="h8p", bufs=3) as h8p,
            tc.tile_pool(name="hq8p", bufs=3) as hq8p,
            tc.tile_pool(name="ab", bufs=2) as ab,
            tc.tile_pool(name="op", bufs=3) as op_,
            tc.tile_pool(name="ps", bufs=3, space="PSUM") as ps,
            tc.tile_pool(name="psO", bufs=5, space="PSUM") as psO,
        ):
            # ---- PE warmup: keeps the PE P-state up during input DMA ----
            warm_in = cp.tile([128, NT], BF16, name="warm_in")
            nc.vector.memset(warm_in, 0.0)
            warm_ps = ps.tile([128, NT], F32, name="warm_ps", tag="ps")
            for _ in range(46):
                nc.tensor.matmul(warm_ps, warm_in[:, :128], warm_in,
                                 start=True, stop=True)
            warm_sb = cp.tile([1, 4], F32, name="warm_sb")
            nc.scalar.copy(warm_sb, warm_ps[:1, :4])

            # ---- phase-A weights: ct=0 slice first ----
            wlow_sb = wp.tile([128, 2 * KL * 128], FP8, name="wlow_sb").rearrange(
                "p (c k m) -> p c k m", c=2, k=KL)
            nc.sync.dma_start(out=wlow_sb[:, 0], in_=wlow_d[:, 0])
            blow_sb = cp.tile([128, 2], F32, name="blow_sb")
            nc.sync.dma_start(out=blow_sb, in_=blow_d)
            wv_sb = wp.tile([128, KL * 256], BF16, name="wv_sb").rearrange(
                "p (k m) -> p k m", k=KL)
            psp_sb = wp.tile([128, KL * MP], BF16, name="psp_sb").rearrange(
                "p (k m) -> p k m", k=KL)

            wq_sb = wp.tile([128, KH * 256], FP8, name="wq_sb").rearrange(
                "p (k m) -> p k m", k=KH)
            wbhb_sb = wp.tile([128, KBH * 2048], BF16, name="wbhb_sb").rearrange(
                "p (k m) -> p k m", k=KBH)
            wbh8_sb = wp.tile([128, KF8 * 2048], FP8, name="wbh8_sb").rearrange(
                "p (k m) -> p k m", k=KF8)
            wcc_sb = wp.tile([128, 2 * CO], FP8, name="wcc_sb").rearrange(
                "p (k m) -> p k m", k=2)
            st = {}  # per-chunk live tiles

            def load_high(c):
                hc = hp.tile([128, KBH * NT], BF16, name="high_c",
                             tag="high_c").rearrange("p (k n) -> p k n", k=KBH)
                nc.sync.dma_start(out=hc, in_=hb_d[:, c])
                hcq = hq8p.tile([128, KF8 * NT], FP8, name="hq8_c",
                                tag="hq8_c").rearrange("p (k n) -> p k n", k=KF8)
                nc.sync.dma_start(out=hcq, in_=hq8_d[:, c])
                st.setdefault(c, {}).update(high=hc, hq8=hcq)

            def load_h8r(c):
                hc8 = h8p.tile([128, KBH * NT], FP8, name="h8r_c",
                               tag="h8r_c").rearrange("p (k n) -> p k n", k=KBH)
                nc.sync.dma_start(out=hc8, in_=h8r_d[:, c])
                st.setdefault(c, {})['h8r'] = hc8

            # ---- phase A: key conv (256 ch, relu) over full spatial ----
            # grid columns: ct-major [ct(2) | gi(24) | gj(24)]
            gridall = gp.tile([128, 2 * 576], F32, name="gridall")

            def conv_ct(nt, ct, low_c):
                cps = ps.tile([128, NT], F32, name="cv_ps", tag="ps")
                for t in range(KL // 2):
                    nc.tensor.matmul(cps, wlow_sb[:, ct, 2 * t:2 * t + 2, :],
                                     low_c[:, 2 * t:2 * t + 2, :],
                                     start=(t == 0), stop=(t == KL // 2 - 1),
                                     perf_mode=DR)
                cv = cvp.tile([128, NT], F32, name="cv", tag="cv")
                nc.scalar.activation(cv, cps, AF.Relu,
                                     bias=blow_sb[:, ct:ct + 1],
                                     scale=1.0 / (SX * SWL))
                # 6 rows x 72 cols -> 2 grid-rows x 24 grid-cols (3x3 cells)
                cvv = cv.rearrange("p (gi ri gj cj) -> p gi gj ri cj",
                                   gi=2, ri=3, gj=24, cj=3)
                gsl = gridall[:, ct * 576 + nt * 48: ct * 576 + (nt + 1) * 48
                              ].rearrange("p (gi gj) -> p gi gj", gi=2)
                for gi in range(2):
                    nc.vector.reduce_sum(gsl[:, gi], cvv[:, gi], axis=AX.XY)

            def extra_dma(nt):
                if nt == 0:
                    nc.sync.dma_start(out=wlow_sb[:, 1], in_=wlow_d[:, 1])
                    nc.sync.dma_start(out=wv_sb, in_=wv_d)
                    nc.sync.dma_start(out=psp_sb, in_=psp_d)
                    bq_sb = cp.tile([128, 2], F32, name="bq_sb")
                    nc.sync.dma_start(out=bq_sb, in_=bq_d)
                    bout_sb = cp.tile([128, 16], F32, name="bout_sb")
                    nc.sync.dma_start(out=bout_sb, in_=bout_d)
                    binv_sb = cp.tile([128, 2 * M], F32, name="binv_sb")
                    nc.sync.dma_start(out=binv_sb, in_=binv_d)
                    cdict['bq'], cdict['bout'], cdict['binv'] = bq_sb, bout_sb, binv_sb
                elif nt == 1:
                    load_high(0)
                    load_h8r(0)
                elif nt == 2:
                    nc.sync.dma_start(out=wq_sb, in_=wq_d)
                    nc.sync.dma_start(out=wbh8_sb[:, 0:2], in_=wbh8_d[:, 0:2])
                elif nt == 3:
                    nc.sync.dma_start(out=wbh8_sb[:, 2:4], in_=wbh8_d[:, 2:4])
                    nc.sync.dma_start(out=wbhb_sb[:, 0], in_=wbhb_d[:, 0])
                elif nt == 4:
                    nc.sync.dma_start(out=wcc_sb, in_=wcc_d)
                    # warmup result store: enqueued only now, after all
                    # latency-critical input DMAs, so Sync never stalls on it
                    nc.sync.dma_start(out=warm_d, in_=warm_sb)
                    nc.sync.dma_start(out=wbh8_sb[:, 4:6], in_=wbh8_d[:, 4:6])
                    nc.sync.dma_start(out=wbhb_sb[:, 1], in_=wbhb_d[:, 1])
                elif nt == 5:
                    nc.sync.dma_start(out=wbh8_sb[:, 6:8], in_=wbh8_d[:, 6:8])
                    nc.sync.dma_start(out=wbhb_sb[:, 2], in_=wbhb_d[:, 2])
                elif nt == 6:
                    nc.sync.dma_start(out=wbh8_sb[:, 8:10], in_=wbh8_d[:, 8:10])
                    nc.sync.dma_start(out=wbhb_sb[:, 3], in_=wbhb_d[:, 3])
                elif nt == 7:
                    nc.sync.dma_start(out=wbhb_sb[:, 4], in_=wbhb_d[:, 4])
                elif nt == 8:
                    nc.sync.dma_start(out=wbhb_sb[:, 5], in_=wbhb_d[:, 5])

            cdict = {}
            low_tiles = {}

            def load_low(nt):
                low_c = lp.tile([128, KL * NT], FP8, name="low_c",
                                tag="low_c").rearrange("p (k n) -> p k n", k=KL)
                nc.sync.dma_start(out=low_c, in_=low_d[:, nt])
                low_tiles[nt] = low_c

            for nt in range(5):
                load_low(nt)

            # ---- value path: small bf16 matmuls on the host-pooled input ----
            v_sc = [pp.tile([128, M], F32, name=f"vsc{t}") for t in range(2)]

            def emit_value():
                for ct in range(2):
                    vps = ps.tile([128, MP], F32, name="v_ps", tag="ps")
                    for t in range(KL):
                        nc.tensor.matmul(vps,
                                         wv_sb[:, t, ct * 128:(ct + 1) * 128],
                                         psp_sb[:, t, :],
                                         start=(t == 0), stop=(t == KL - 1))
                    nc.vector.tensor_mul(v_sc[ct], vps[:, 0:110],
                                         cdict['binv'][:, M:2 * M])

            def emit_q(c):
                # query conv reads the GPTQ fp8 tiles (0..KF8) plus an RTN
                # fp8 copy of the bf16 tiles -- no separate full-16 tensor
                if 'q' in st[c]:
                    return
                hcq, hr = st[c]['hq8'], st[c]['h8r']
                q_sb = ab.tile([128, 2 * NT], FP8, name="q_sb",
                               tag="q_sb").rearrange("p (t n) -> p t n", t=2)
                for qt in range(2):
                    qps = ps.tile([128, NT], F32, name="q_ps", tag="ps")
                    for t in range(KF8 // 2):
                        nc.tensor.matmul(qps,
                                         wq_sb[:, 2 * t:2 * t + 2,
                                               qt * 128:(qt + 1) * 128],
                                         hcq[:, 2 * t:2 * t + 2, :],
                                         start=(t == 0), stop=False,
                                         perf_mode=DR)
                    for t in range(KBH // 2):
                        nc.tensor.matmul(qps,
                                         wq_sb[:, KF8 + 2 * t:KF8 + 2 * t + 2,
                                               qt * 128:(qt + 1) * 128],
                                         hr[:, 2 * t:2 * t + 2, :],
                                         start=False, stop=(t == KBH // 2 - 1),
                                         perf_mode=DR)
                    nc.scalar.activation(q_sb[:, qt], qps, AF.Relu,
                                         bias=cdict['bq'][:, qt:qt + 1],
                                         scale=0.0625 * SQ8 / (SX * SWQ))
                st[c]['q'] = q_sb

            # ---- chunk-0 fused high-parts run inside phase A; each finished
            # PSUM accumulation is spilled to SBUF bf16 so the banks rotate.
            # After the attention chain, a cheap bf16-identity matmul reloads
            # each spill and the wcc ctx pair finishes the group. ----
            acc0 = wp.tile([128, 16 * NT], BF16, name="acc0").rearrange(
                "p (ct n) -> p ct n", ct=16)

            def c0_high(cts):
                hc0, hcq0 = st[0]['high'], st[0]['hq8']
                tiles = {}
                for ct in cts:
                    ops = psO.tile([128, NT], F32, name="o_ps", tag="psO")
                    for t in range(KF8 // 2):
                        nc.tensor.matmul(ops,
                                         wbh8_sb[:, 2 * t:2 * t + 2,
                                                 ct * 128:(ct + 1) * 128],
                                         hcq0[:, 2 * t:2 * t + 2, :],
                                         start=(t == 0), stop=False,
                                         perf_mode=DR)
                    tiles[ct] = ops
                for ct in cts:
                    ops = tiles[ct]
                    for k in range(KBH):
                        nc.tensor.matmul(ops,
                                         wbhb_sb[:, k, ct * 128:(ct + 1) * 128],
                                         hc0[:, k, :], start=False,
                                         stop=(k == KBH - 1))
                    nc.scalar.copy(acc0[:, ct], ops)

            def c0_fin(cts):
                tiles = {}
                for ct in cts:
                    ops = psO.tile([128, NT], F32, name="o_ps", tag="psO")
                    nc.tensor.matmul(ops, ident_bf, acc0[:, ct],
                                     start=True, stop=False)
                    tiles[ct] = ops
                for ct in cts:
                    ops = tiles[ct]
                    nc.tensor.matmul(ops, wcc_sb[:, :, ct * 128:(ct + 1) * 128],
                                     st[0]['ctx'], start=False, stop=True,
                                     perf_mode=DR)
                    emit_out(0, ct, ops)

            for nt in range(NTILES_A):
                if nt + 5 < NTILES_A:
                    load_low(nt + 5)
                low_c = low_tiles.pop(nt)
                if nt == 0:
                    conv_ct(0, 0, low_c)
                    extra_dma(0)
                    ident = cp.tile([128, 128], F32, name="ident")
                    masks.make_identity(nc, ident)
                    ident_bf = cp.tile([128, 128], BF16, name="ident_bf")
                    masks.make_identity(nc, ident_bf)
                    # 1/16 here folds the SCC=16 ctx scale into the softmax
                    # normalizer: r = 16/sum(E)
                    ones110 = cp.tile([110, 1], BF16, name="ones110")
                    nc.vector.memset(ones110, 1.0 / SCC)
                    ones1 = cp.tile([1, 128], BF16, name="ones1")
                    nc.vector.memset(ones1, 1.0)
                    conv_ct(nt, 1, low_c)
                    continue
                extra_dma(nt)
                conv_ct(nt, 0, low_c)
                conv_ct(nt, 1, low_c)
                if nt == 3:
                    emit_value()
                elif nt == 4:
                    emit_q(0)
                elif nt == 8:
                    c0_high(range(0, 2))
                elif nt == 9:
                    c0_high(range(2, 4))
                    load_h8r(1)
                elif nt == 10:
                    c0_high(range(4, 6))
                    load_high(1)
                elif nt == 11:
                    c0_high(range(6, 8))
                    load_h8r(2)
            c0_high(range(8, 12))
            load_high(2)
            c0_high(range(12, 16))
            bq_sb, bout_sb, binv_sb = cdict['bq'], cdict['bout'], cdict['binv']

            # ---- stage-2 pooling: grid -> 110 pooled key sums per tile ----
            pooled = [pp.tile([128, M], F32, name=f"pooled{t}") for t in range(2)]

            def stage2(t):
                g = gridall[:, t * 576:(t + 1) * 576]
                nc.vector.reduce_sum(pooled[t][:, 0:1], g, axis=AX.X)
                for s, off in ((3, 1), (6, 10), (8, 46)):
                    cs = 24 // s
                    gv = g.rearrange("p (bi ci bj cj) -> p bi bj ci cj",
                                     bi=s, ci=cs, bj=s, cj=cs)
                    for bi in range(s):
                        nc.vector.reduce_sum(
                            pooled[t][:, off + bi * s: off + (bi + 1) * s],
                            gv[:, bi], axis=AX.XY)

            k8 = pp.tile([128, 2 * MP], FP8, name="k8").rearrange(
                "p (t m) -> p t m", t=2)
            for t in range(2):
                stage2(t)
                nc.vector.tensor_mul(k8[:, t, 0:110], pooled[t], binv_sb[:, 0:M])
            VT = [pp.tile([110, 128], BF16, name=f"VT{t}") for t in range(2)]

            # ---- attention stages ----
            def emit_sim(c):
                if 'E' in st[c]:
                    return
                sim_ps = ps.tile([110, NT], F32, name="sim_ps", tag="ps")
                nc.tensor.matmul(sim_ps, k8[:, :, 0:110], st[c]['q'],
                                 start=True, stop=True, perf_mode=DR)
                E = ab.tile([110, NT], BF16, name="E", tag="E")
                nc.scalar.activation(E, sim_ps, AF.Exp, scale=1.0 / (SK8 * SQ8))
                st[c]['E'] = E

            def emit_s(c):
                if 'r' in st[c]:
                    return
                s_ps = ps.tile([1, NT], F32, name="s_ps", tag="ps")
                nc.tensor.matmul(s_ps, ones110, st[c]['E'], start=True, stop=True)
                s_sb = ab.tile([1, NT], F32, name="s_sb", tag="s_sb")
                nc.vector.tensor_copy(s_sb, s_ps)
                r_sb = ab.tile([1, NT], F32, name="r_sb", tag="r_sb")
                nc.vector.reciprocal_approx_fast(r_sb, s_sb)
                r_bf = ab.tile([1, NT], BF16, name="r_bf", tag="r_bf")
                nc.vector.tensor_copy(r_bf, r_sb)
                st[c]['r'] = r_bf

            def emit_rbc(c):
                rbc_ps = ps.tile([128, NT], F32, name="rbc_ps", tag="ps")
                nc.tensor.matmul(rbc_ps, ones1, st[c]['r'], start=True, stop=True)
                rbc_sb = ab.tile([128, NT], F32, name="rbc_sb", tag="rbc_sb")
                nc.vector.tensor_copy(rbc_sb, rbc_ps)
                st[c]['rbc'] = rbc_sb

            def emit_ctx(c):
                ctx8 = ab.tile([128, 2 * NT], FP8, name="ctx8",
                               tag="ctx8").rearrange("p (t n) -> p t n", t=2)
                for t in range(2):
                    cps = ps.tile([128, NT], F32, name="ctx_ps", tag="ps")
                    nc.tensor.matmul(cps, VT[t], st[c]['E'], start=True, stop=True)
                    nc.vector.tensor_mul(ctx8[:, t], cps, st[c]['rbc'])
                st[c]['ctx'] = ctx8

            def emit_out(c, ct, ops):
                j = ct % 4
                if j == 0:
                    st[c]['o4'] = op_.tile([128, 4 * NT], BF16, name="o4",
                                           tag="o4").rearrange(
                        "p (j n) -> p j n", j=4)
                o4 = st[c]['o4']
                nc.vector.tensor_scalar(o4[:, j], ops, 1.0 / SPS,
                                        bout_sb[:, ct:ct + 1],
                                        ALU.mult, ALU.add)
                if j == 3:
                    nc.sync.dma_start(
                        out=out_d[:, c, ct - 3:ct + 1, :], in_=o4)

            def emit_fused(c, cts):
                # grouped: all fp8-DR parts (incl. the wcc ctx pair, which is
                # ready in steady state) back-to-back, then all bf16 parts --
                # one bf16<->fp8 PE mode switch per group instead of per ct
                hc, hcq, ctx = st[c]['high'], st[c]['hq8'], st[c]['ctx']
                for ct in cts:
                    ops = psO.tile([128, NT], F32, name="o_ps", tag="psO")
                    nc.tensor.matmul(ops, wcc_sb[:, :, ct * 128:(ct + 1) * 128],
                                     ctx, start=True, stop=False, perf_mode=DR)
                    for t in range(KF8 // 2):
                        nc.tensor.matmul(ops,
                                         wbh8_sb[:, 2 * t:2 * t + 2,
                                                 ct * 128:(ct + 1) * 128],
                                         hcq[:, 2 * t:2 * t + 2, :],
                                         start=False, stop=False, perf_mode=DR)
                    st[c][f'ops{ct}'] = ops
                for ct in cts:
                    ops = st[c].pop(f'ops{ct}')
                    for k in range(KBH):
                        nc.tensor.matmul(ops,
                                         wbhb_sb[:, k, ct * 128:(ct + 1) * 128],
                                         hc[:, k, :], start=False,
                                         stop=(k == KBH - 1))
                    emit_out(c, ct, ops)

            # ---- chunk-0 attention chain; chunk-1 stages and the chunk-0
            # fin groups fill its serial latencies ----
            emit_sim(0)
            for t in range(2):
                tp = ps.tile([110, 128], F32, name="vt_ps", tag="ps")
                nc.tensor.transpose(tp, v_sc[t], ident)
                nc.scalar.copy(VT[t], tp)
            emit_s(0)
            emit_q(1)            # hides the reciprocal chain of chunk 0
            emit_rbc(0)
            emit_sim(1)
            emit_ctx(0)
            c0_fin(range(0, 4))
            emit_s(1)
            c0_fin(range(4, 8))
            emit_rbc(1)
            emit_q(2)
            c0_fin(range(8, 12))
            emit_ctx(1)
            c0_fin(range(12, 16))
            del st[0]

            # ---- steady pipeline (q runs one chunk ahead of the chain) ----
            for c in range(1, NCH_B):
                n = c + 1
                if c + 2 < NCH_B:
                    load_h8r(c + 2)
                    load_high(c + 2)
                if n < NCH_B:
                    if n + 1 < NCH_B:
                        emit_q(n + 1)
                    emit_fused(c, range(0, 4))
                    emit_sim(n)
                    emit_fused(c, range(4, 8))
                    emit_s(n)
                    emit_fused(c, range(8, 12))
                    emit_rbc(n)
                    emit_ctx(n)
                    emit_fused(c, range(12, 16))
                else:
                    for g in range(0, 16, 4):
                        emit_fused(c, range(g, g + 4))
                del st[c]

    nc.compile()
    return nc


def _pack_tiles(a2d, ktiles):
    """(ktiles*128, F) -> [128, ktiles, F] contiguous."""
    k128, F = a2d.shape
    assert k128 == ktiles * 128
    return np.ascontiguousarray(a2d.reshape(ktiles, 128, F).transpose(1, 0, 2))


def _chunk_major(a2d, ktiles, dtype):
    """(ktiles*128, NL) -> [128, NCH_B, ktiles, NT] contiguous."""
    k128, nl = a2d.shape
    a = a2d.reshape(ktiles, 128, NCH_B, NT)
    return np.ascontiguousarray(a.transpose(1, 2, 0, 3)).astype(dtype)


def _bf16(a):
    return np.ascontiguousarray(a).astype(BF)


def _fp8(a, scale):
    x = np.asarray(a, np.float32) * scale
    return np.clip(x, -224.0, 224.0).astype(F8)


def _q_fp8(v, scale, clip=224.0):
    return (np.clip(v * scale, -clip, clip).astype(F8).astype(np.float32)
            / np.float32(scale))


def _q_bf16(v):
    return v.astype(BF).astype(np.float32)


def _gptq_mixed(Wf, Hd, nf8, scale8, blocksize=128, damp=0.01):
    """Round Wf (K, O) rows [0,nf8) to the fp8 grid (at scale8) and the rest
    to bf16, sequentially compensating each row's rounding error via the
    remaining rows (GPTQ). Hd: damped XX^T (K, K) f64. Returns f32 raw-domain
    quantized values."""
    K, O = Wf.shape
    W = np.array(Wf, np.float32)
    L = np.linalg.cholesky(np.linalg.inv(Hd))
    U = np.ascontiguousarray(L.T, np.float32)
    Q = np.empty_like(W)
    for i0 in range(0, nf8, blocksize):
        i1 = min(i0 + blocksize, nf8)
        Wb = W[i0:i1].copy()
        Eb = np.empty_like(Wb)
        for j in range(i1 - i0):
            k = i0 + j
            w = Wb[j]
            qv = _q_fp8(w, scale8)
            Q[k] = qv
            e = (w - qv) / U[k, k]
            Eb[j] = e
            if j + 1 < i1 - i0:
                Wb[j + 1:] -= np.outer(U[k, i0 + j + 1:i1], e)
        if i1 < K:
            W[i1:] -= U[i0:i1, i1:].T @ Eb
    Q[nf8:] = _q_bf16(W[nf8:])
    return Q


def _damp(Hf32):
    Hd = Hf32.astype(np.float64)
    Hd[np.diag_indices(Hd.shape[0])] += np.mean(np.diag(Hd)) * 0.01
    return Hd


def _prep_consts(inputs):
    f64 = {k: np.asarray(v, np.float64) for k, v in inputs.items()}
    sk = f64['gk'] / np.sqrt(f64['vk'] + EPS)
    sq = f64['gq'] / np.sqrt(f64['vq'] + EPS)
    sf = f64['gf'] / np.sqrt(f64['vf'] + EPS)

    wk_f = f64['wk'] * sk[:, None]
    bk_f = (f64['bk'] - f64['mk']) * sk + f64['bek']
    wq_f = f64['wq'] * sq[:, None]
    bq_f = ((f64['bq'] - f64['mq']) * sq + f64['beq']) / 16.0 * SQ8

    wf_s = f64['wf'] * sf[:, None]
    A = wf_s[:, :CO]          # context part (2048, 2048)
    Bh = wf_s[:, CO:]         # high part (2048, 2048)
    Wcc = A @ f64['ww']       # (2048, 256)
    # bv passes through the softmax-weighted value average unchanged
    # (weights sum to 1), so it folds into the output bias via Wcc.
    bout = (A @ f64['bw'] + (f64['bf'] - f64['mf']) * sf + f64['bef']
            + Wcc @ f64['bv'])

    binsize = np.concatenate(
        [np.full(s * s, float((H // s) * (W // s))) for s in PSP])  # (110,)
    s_m = 8.0 / np.sqrt(binsize)          # per-column scale of psp sums
    binv2 = np.concatenate([SK8 / binsize, 1.0 / ((s_m / 8.0) * binsize)])

    return {
        'wlow': _fp8(np.ascontiguousarray(
            wk_f.T.reshape(KL, 128, 2, 128).transpose(1, 2, 0, 3)), SWL),
        'wv': _bf16(_pack_tiles(np.ascontiguousarray(f64['wv'].T), KL)),
        'wq': _fp8(_pack_tiles(np.ascontiguousarray(wq_f.T), KH), SWQ),
        'wcc': _fp8(_pack_tiles(np.ascontiguousarray(Wcc.T), 2), SPS / SCC),
        'blow': np.ascontiguousarray(bk_f.reshape(2, 128).T, dtype=np.float32),
        'bq': np.ascontiguousarray(bq_f.reshape(2, 128).T, dtype=np.float32),
        'bout': np.ascontiguousarray(bout.reshape(16, 128).T, dtype=np.float32),
        'binv': np.ascontiguousarray(
            np.broadcast_to(binv2, (128, 2 * M)), dtype=np.float32),
        '_s_m': s_m,
        '_Wfull': np.ascontiguousarray(Bh.T, np.float32),   # (CH, CO)
    }


def _make_in_maps(inputs):
    consts = _prep_consts(inputs)
    s_m = consts.pop('_s_m')
    Wfull = consts.pop('_Wfull')
    low_feats = np.asarray(inputs['low_feats'], np.float32)
    high_feats = np.asarray(inputs['high_feats'], np.float32)

    # host-side spatial-pyramid sums of the low features (the value path's
    # pooling commutes with its 1x1 conv), bf16 with per-column scales
    lowr = low_feats.reshape(B, CL, H, W).astype(np.float64)
    psums = np.concatenate(
        [lowr.reshape(B, CL, s, H // s, s, W // s).sum(axis=(3, 5))
         .reshape(B, CL, s * s) for s in PSP], axis=2)       # (B, CL, 110)
    pspq = np.zeros((B, CL, MP), np.float64)
    pspq[:, :, :M] = psums * (s_m / 8.0)

    in_maps = []
    for c in range(NCORES):
        b, s = c // 2, c % 2
        low2d = low_feats[b].reshape(CL, N)
        X = np.ascontiguousarray(
            high_feats[b, :, s * HL:(s + 1) * HL, :].reshape(CH, NL))

        # GPTQ-mixed rounding: W vs this core's RTN-quantized X, then X vs
        # the quantized W; bf16 rows absorb the fp8 rows' rounding error.
        Xq_rtn = np.concatenate([_q_fp8(X[:NF8], SX), _q_bf16(X[NF8:])])
        Wg = _gptq_mixed(Wfull, _damp(Xq_rtn @ Xq_rtn.T), NF8, SWH)
        Xg = _gptq_mixed(X, _damp(Wg @ Wg.T), NF8, SX)

        m = dict(consts)
        m['low'] = np.ascontiguousarray(
            _fp8(_pack_tiles(low2d, KL), SX)          # [128, KL, N]
            .reshape(128, KL, NTILES_A, NT).transpose(0, 2, 1, 3))
        m['h8r'] = _chunk_major(_fp8(X[NF8:], SX), KBH, F8)
        m['hq8'] = _chunk_major(_fp8(Xg[:NF8], SX), KF8, F8)
        m['hb'] = _chunk_major(Xg[NF8:], KBH, BF)
        m['wbh8'] = _fp8(_pack_tiles(Wg[:NF8], KF8), SWH)
        m['wbhb'] = _bf16(_pack_tiles(Wg[NF8:] * SPS, KBH))
        m['psp'] = _bf16(_pack_tiles(pspq[b], KL))
        in_maps.append(m)
    return in_maps


def kernel(**inputs):
    global LAST_RESULTS
    from concourse.bass_utils import run_bass_kernel_spmd

    if 'nc' not in _CACHE:
        _CACHE['nc'] = _build_graph()
    nc = _CACHE['nc']

    res = run_bass_kernel_spmd(nc, _make_in_maps(inputs), list(range(NCORES)))
    LAST_RESULTS = res

    out = np.empty((B, CO, H, W), np.float32)
    for c in range(NCORES):
        b, s = c // 2, c % 2
        o = np.asarray(res.results[c]['out'], np.float32)  # [128, 6, 16, NT]
        o = o.transpose(2, 0, 1, 3).reshape(CO, HL, W)
        out[b, :, s * HL:(s + 1) * HL, :] = o
    return out
